# revision 8
# baseline (speedup 1.0000x reference)
"""Causal self-attention (T=2048, C=1024, H=16) on 8 Trainium2 NeuronCores.

Tensor-parallel over heads: each core owns 2 heads (wqkv row-shard), computes
qkv + attention for its heads, then multiplies its 128 attention rows by the
full projection matrix to produce a PARTIAL output (its heads' contribution to
all 1024 output columns). The host sums the 8 partials and adds proj_b — the
"all-reduce after proj" runs as part of the host-side gather/unshard, so the
device pipeline needs no cross-core synchronization at all.

v3 (vs the fp32r baseline at 165us):
  - all matmul operands in bf16 (1 cycle/row on the PE, FWL weight loads,
    half the HBM + SBUF traffic); PSUM accumulation stays fp32.
  - v is computed directly in [token, dim] layout (lhsT=x tile, rhs=wv
    columns), eliminating the 16 PE transposes + 32 DVE repack copies.
  - partial output written as fp16 (half the output DMA); host sums in fp32.
  - reciprocal_approx_fast for the softmax denominators (plain DVE
    reciprocal measured 3.3us per [64,512] tile — 27us total).
  - software-pipelined emission: per chunk g the engine programs are
    attention(g) -> normalize(g) -> qkv(g+1) -> proj(g), so the PE FIFO
    never stalls at a chunk tail waiting for DVE (the proj matmuls sit
    behind qkv(g+1)), and DVE never stalls waiting for PE (normalize(g)
    sits before the g+1 bias-adds). All proj-psum evacuations are on DVE:
    putting any on ACT would block the strict-FIFO exp stream.
  - DMA issue cost (~0.6us per dma_start on the Sync sequencer) split
    across the Sync and GpSimd queues.
  - PE warm-up matmuls on the weight tile while x streams in, so the HAM
    clock gate reaches 8/8 before the first real matmul.
  - exp() is safe without max-subtraction: |scores| < 4 for this problem.

Layout notes (per core c, heads 2c and 2c+1):
  - xT   [1024, 2048]  x transposed (shared by all cores), bf16
  - wT   [1024, 384]   wqkv rows for (k,q,v) of this core's heads, transposed;
                       q-rows pre-scaled by 1/sqrt(64)=0.125 (exact)
  - qT/kT [128, 2048] in SBUF: rows = 2 heads x 64 dims, cols = tokens
  - v_sb [128, 16, 2, 65]: tokens on partitions, per token-tile / head the 64
    v dims plus a ones column (col 64) that makes row 64 of the AV psum
    accumulate the softmax denominator — no cross-partition reductions needed.
  - v bias is added after normalization (softmax rows sum to 1, so adding
    b_v to every v row adds exactly b_v to the attention output).
"""

from collections import deque

import numpy as np
import ml_dtypes

import concourse.bass as bass
import concourse.mybir as mybir
import concourse.tile as tile
from concourse import bacc
from concourse import bass_utils

T = 2048
C = 1024
H = 16
D = 64
N_CORES = 8
P = 128
NT = T // P          # 16 token tiles
NG = T // 512        # 4 column chunks of 512
NO = C // P          # 8 contraction subtiles

F32 = mybir.dt.float32
BF16 = mybir.dt.bfloat16
F16 = mybir.dt.float16
BF16_NP = ml_dtypes.bfloat16


def _build():
    nc = bacc.Bacc("TRN2", target_bir_lowering=False, debug=False,
                   num_devices=N_CORES)

    xT = nc.dram_tensor("xT", [C, T], BF16, kind="ExternalInput").ap()
    wT = nc.dram_tensor("wT", [C, 3 * P], BF16, kind="ExternalInput").ap()
    bqkv = nc.dram_tensor("bqkv", [P, 3], F32, kind="ExternalInput").ap()
    # pwl[i, o]: proj_w columns for this core's 128 attn rows, transposed
    pwl = nc.dram_tensor("pwl", [P, C], BF16, kind="ExternalInput").ap()
    mask01 = nc.dram_tensor("mask01", [P, 2, P], BF16, kind="ExternalInput").ap()
    # partial output: [1024 out-cols (as 8x128), 2048 tokens], fp16
    outP = nc.dram_tensor("outP", [C, T], F16, kind="ExternalOutput").ap()
    outP3 = outP.rearrange("(o p) t -> p o t", p=P)

    xT3 = xT.rearrange("(o p) t -> p o t", p=P)      # [128, 8, 2048]
    wT3 = wT.rearrange("(o p) j -> p o j", p=P)      # [128, 8, 384]

    # wqkv section order in wT columns: k, q, v
    JK, JQ, JV = 0, 1, 2

    with tile.TileContext(nc) as tc:
        with (
            tc.tile_pool(name="const", bufs=1) as constp,
            tc.tile_pool(name="big", bufs=1) as bigp,
            tc.tile_pool(name="work", bufs=4) as workp,
            tc.tile_pool(name="small", bufs=4) as smallp,
            tc.tile_pool(name="attn", bufs=2) as attnp,
            tc.tile_pool(name="outp", bufs=2) as outp,
            tc.tile_pool(name="ps_mm", bufs=2, space="PSUM") as ps_mm,
            tc.tile_pool(name="ps_sc", bufs=2, space="PSUM") as ps_sc,
            tc.tile_pool(name="ps_at", bufs=2, space="PSUM") as ps_at,
        ):
            # ---- PE warm-up on a memset tile: no DMA dependency, so the
            # HAM clock gate reaches 8/8 while the inputs are still in
            # flight and the first real matmul runs at 2.4 GHz ----
            warm_sb = constp.tile([P, 512], BF16, name="warm")
            nc.vector.memset(warm_sb[:], 1.0)
            wu = ps_sc.tile([P, 2, 512], F32, name="sc")
            for r in range(14):
                nc.tensor.matmul(wu[:, 0, :],
                                 lhsT=warm_sb[:, 0:P],
                                 rhs=warm_sb[:],
                                 start=True, stop=True)

            # ---- inputs; one batched dma_start per chunk (every dma_start
            # costs ~0.6us of sequencer time), spread over the Sync and
            # GpSimd queues; the first chunk of x is split across both
            # queues so qkv(0) can start earliest ----
            wT_sb = constp.tile([P, NO, 3 * P], BF16, name="wT")
            x_sb = bigp.tile([P, NO, T], BF16, name="x")
            bq_sb = constp.tile([P, 3], F32, name="bqkv")
            mask_sb = constp.tile([P, 2, P], BF16, name="mask")
            ones_sb = constp.tile([1, D], BF16, name="ones")
            nc.vector.memset(ones_sb[:], 1.0)
            pwl_sb = constp.tile([P, NO, P], BF16, name="pwl")
            nc.sync.dma_start(wT_sb[:], wT3[:])
            nc.gpsimd.dma_start(x_sb[:, 0:NO // 2, 0:512],
                                xT3[:, 0:NO // 2, 0:512])
            nc.sync.dma_start(x_sb[:, NO // 2:, 0:512],
                              xT3[:, NO // 2:, 0:512])
            nc.gpsimd.dma_start(bq_sb[:], bqkv)
            nc.gpsimd.dma_start(x_sb[:, :, 512:1024], xT3[:, :, 512:1024])
            nc.sync.dma_start(mask_sb[:], mask01)
            nc.sync.dma_start(x_sb[:, :, 1024:1536], xT3[:, :, 1024:1536])
            nc.gpsimd.dma_start(x_sb[:, :, 1536:2048], xT3[:, :, 1536:2048])
            nc.sync.dma_start(pwl_sb[:],
                              pwl.rearrange("p (o q) -> p o q", q=P))

            kT_sb = bigp.tile([P, T], BF16, name="kT")
            qT_sb = bigp.tile([P, T], BF16, name="qT")
            v_sb = bigp.tile([P, NT, 2, D + 1], BF16, name="v")
            nc.vector.memset(v_sb[:, :, :, D], 1.0)

            fillers = deque()

            def qkv_fillers(g):
                """Queue chunk g's qkv work as PE filler thunks (consumed
                inside the previous chunk's attention j-loop, where the PE
                otherwise idles waiting on ACT exp)."""
                cols = slice(g * 512, (g + 1) * 512)
                state = {}

                def kq_mm(j, dst, o):
                    def f():
                        if o == 0:
                            state[j] = ps_mm.tile([P, 512], F32, name="mm")
                        nc.tensor.matmul(
                            state[j][:],
                            lhsT=wT_sb[:, o, j * P:(j + 1) * P],
                            rhs=x_sb[:, o, cols],
                            start=(o == 0), stop=(o == NO - 1),
                        )
                        if o == NO - 1:
                            nc.vector.tensor_scalar_add(dst[:, cols],
                                                        state[j][:],
                                                        bq_sb[:, j:j + 1])
                    return f

                def v_mm(tt, o):
                    def f():
                        if tt == 0 and o == 0:
                            state[JV] = ps_mm.tile([P, 4, 2, D], F32,
                                                   name="mm")
                        t0 = g * 512 + tt * P
                        nc.tensor.matmul(
                            state[JV][:, tt, :, :],
                            lhsT=x_sb[:, o, t0:t0 + P],
                            rhs=wT_sb[:, o, JV * P:(JV + 1) * P],
                            start=(o == 0), stop=(o == NO - 1),
                        )
                        if tt == 3 and o == NO - 1:
                            nc.vector.tensor_copy(
                                out=v_sb[:, 4 * g:4 * g + 4, :, 0:D],
                                in_=state[JV][:])
                    return f

                for j, dst in ((JK, kT_sb), (JQ, qT_sb)):
                    for o in range(NO):
                        fillers.append(kq_mm(j, dst, o))
                for tt in range(4):
                    for o in range(NO):
                        fillers.append(v_mm(tt, o))

            def qkv(g):
                qkv_fillers(g)
                while fillers:
                    fillers.popleft()()

            def attention(g):
                ats = [ps_at.tile([P, 512], F32, name="at") for _ in range(2)]
                for j in range(4 * g + 4):
                    for _ in range(3):
                        if fillers:
                            fillers.popleft()()
                    t0 = 512 * g if j < 4 * g else P * j
                    w_ = 512 * (g + 1) - t0
                    sc2 = ps_sc.tile([P, 2, 512], F32, name="sc")
                    for h in range(2):
                        nc.tensor.matmul(
                            sc2[:, h, 0:w_],
                            lhsT=kT_sb[h * D:(h + 1) * D, j * P:(j + 1) * P],
                            rhs=qT_sb[h * D:(h + 1) * D, t0:t0 + w_],
                            start=True, stop=True,
                        )
                    e2 = workp.tile([P, 2, 512], BF16, name="e2")
                    if w_ == 512:
                        nc.scalar.activation(e2[:], sc2[:],
                                             mybir.ActivationFunctionType.Exp)
                    else:
                        for h in range(2):
                            nc.scalar.activation(
                                e2[:, h, 0:w_], sc2[:, h, 0:w_],
                                mybir.ActivationFunctionType.Exp)
                    if j >= 4 * g:
                        # zero the strictly-upper (t<s) part of the diag block
                        nc.vector.tensor_mul(out=e2[:, :, 0:P],
                                             in0=e2[:, :, 0:P],
                                             in1=mask_sb[:])
                    for h in range(2):
                        nc.tensor.matmul(
                            ats[h][:D + 1, t0 - 512 * g:512],
                            lhsT=v_sb[:, j, h, :],
                            rhs=e2[:, h, 0:w_],
                            start=(j == 0), stop=(j == 4 * g + 3),
                        )
                return ats

            def normalize(g, ats):
                # rows 0..63 of ats[h] = unnormalized attnT, row 64 = denom
                attn_sb = attnp.tile([P, 512], BF16, name="attn")
                for h in range(2):
                    at = ats[h]
                    rs = smallp.tile([1, 512], BF16, name="rs")
                    nc.vector.tensor_copy(out=rs[:], in_=at[D:D + 1, :])
                    rb = ps_mm.tile([P, 512], F32, name="mm")
                    nc.tensor.matmul(rb[:D, :], lhsT=ones_sb[:], rhs=rs[:],
                                     start=True, stop=True)
                    rr = smallp.tile([D, 512], F32, name="rr")
                    nc.vector.reciprocal_approx_fast(out=rr[:], in_=rb[:D, :])
                    nc.vector.tensor_mul(out=attn_sb[h * D:(h + 1) * D, :],
                                         in0=at[0:D, :], in1=rr[:])
                # v bias (softmax rows sum to 1 -> plain add post-normalize)
                nc.vector.tensor_scalar_add(attn_sb[:], attn_sb[:],
                                            bq_sb[:, JV:JV + 1])
                return attn_sb

            def proj_fillers(g, attn_sb, last):
                cols = slice(g * 512, (g + 1) * 512)
                state = {}

                def pmm(ot):
                    def f():
                        if ot == 0:
                            state["ob"] = outp.tile([P, NO, 512], F16,
                                                    name="ob")
                        psp = ps_mm.tile([P, 512], F32, name="mm")
                        nc.tensor.matmul(psp[:], lhsT=pwl_sb[:, ot, :],
                                         rhs=attn_sb[:],
                                         start=True, stop=True)
                        ob = state["ob"]
                        # on the last chunk ACT has no more exps: split the
                        # evacuation between DVE and ACT to shorten the tail
                        if last and ot % 2 == 1:
                            nc.scalar.copy(ob[:, ot, :], psp[:])
                        else:
                            nc.vector.tensor_copy(out=ob[:, ot, :], in_=psp[:])
                        if ot == NO // 2 - 1:
                            nc.sync.dma_start(outP3[:, 0:NO // 2, cols],
                                              ob[:, 0:NO // 2, :])
                        elif ot == NO - 1:
                            nc.gpsimd.dma_start(outP3[:, NO // 2:, cols],
                                                ob[:, NO // 2:, :])
                    return f

                for ot in range(NO):
                    fillers.append(pmm(ot))

            # ---- software-pipelined chunk loop: chunk g's attention
            # j-loop absorbs proj(g-1) and qkv(g+1) as PE fillers ----
            qkv(0)
            for g in range(NG):
                if g + 1 < NG:
                    qkv_fillers(g + 1)
                ats = attention(g)
                while fillers:
                    fillers.popleft()()
                attn_sb = normalize(g, ats)
                proj_fillers(g, attn_sb, last=(g + 1 == NG))
            while fillers:
                fillers.popleft()()

    nc.compile()
    return nc


_NC = None
LAST_RESULT = None


def _get_nc():
    global _NC
    if _NC is None:
        _NC = _build()
    return _NC


def _prep_inputs(x, wqkv_w, wqkv_b, proj_w, proj_b):
    x = np.asarray(x, np.float32)
    wqkv_w = np.asarray(wqkv_w, np.float32)
    wqkv_b = np.asarray(wqkv_b, np.float32)
    proj_w = np.asarray(proj_w, np.float32)

    scale = np.float32(1.0 / np.sqrt(D))  # 0.125 exactly
    xT = np.ascontiguousarray(x.T).astype(BF16_NP)
    mask1 = np.triu(np.ones((P, P), np.float32))
    mask = np.ascontiguousarray(
        np.broadcast_to(mask1[:, None, :], (P, 2, P))).astype(BF16_NP)

    in_maps = []
    for c in range(N_CORES):
        qs = slice(P * c, P * (c + 1))
        ks = slice(C + P * c, C + P * (c + 1))
        vs = slice(2 * C + P * c, 2 * C + P * (c + 1))
        # column order in wT: k, q, v (q-rows pre-scaled)
        w_c = np.concatenate(
            [wqkv_w[ks], wqkv_w[qs] * scale, wqkv_w[vs]], axis=0)  # [384, 1024]
        b_c = np.concatenate(
            [wqkv_b[ks], wqkv_b[qs] * scale, wqkv_b[vs]])          # [384]
        in_maps.append({
            "xT": xT,
            "wT": np.ascontiguousarray(w_c.T).astype(BF16_NP),
            "bqkv": np.ascontiguousarray(b_c.reshape(3, P).T, dtype=np.float32),
            # proj_w columns for this core's attn rows, transposed -> [128, 1024]
            "pwl": np.ascontiguousarray(proj_w[:, qs].T).astype(BF16_NP),
            "mask01": mask,
        })
    return in_maps


def kernel(x, wqkv_w, wqkv_b, proj_w, proj_b):
    global LAST_RESULT
    nc = _get_nc()
    in_maps = _prep_inputs(x, wqkv_w, wqkv_b, proj_w, proj_b)
    res = bass_utils.run_bass_kernel_spmd(nc, in_maps,
                                          core_ids=list(range(N_CORES)))
    LAST_RESULT = res
    # unshard: the partials are sum-sharded over cores; reduce, transpose,
    # and apply the projection bias once.
    acc = res.results[0]["outP"].astype(np.float32)
    for c in range(1, N_CORES):
        acc = acc + res.results[c]["outP"].astype(np.float32)
    out = acc.T + np.asarray(proj_b, np.float32)[None, :]
    return np.ascontiguousarray(out).astype(np.float32)


# revision 9
# speedup vs baseline: 1.0355x; 1.0355x over previous
"""Causal self-attention (T=2048, C=1024, H=16) on 8 Trainium2 NeuronCores.

Tensor-parallel over heads: each core owns 2 heads (wqkv row-shard), computes
qkv + attention for its heads, then multiplies its 128 attention rows by the
full projection matrix to produce a PARTIAL output (its heads' contribution to
all 1024 output columns). The host sums the 8 partials and adds proj_b — the
"all-reduce after proj" runs as part of the host-side gather/unshard, so the
device pipeline needs no cross-core synchronization at all.

v3 (vs the fp32r baseline at 165us):
  - all matmul operands in bf16 (1 cycle/row on the PE, FWL weight loads,
    half the HBM + SBUF traffic); PSUM accumulation stays fp32.
  - v is computed directly in [token, dim] layout (lhsT=x tile, rhs=wv
    columns), eliminating the 16 PE transposes + 32 DVE repack copies.
  - partial output written as fp16 (half the output DMA); host sums in fp32.
  - reciprocal_approx_fast for the softmax denominators (plain DVE
    reciprocal measured 3.3us per [64,512] tile — 27us total).
  - software-pipelined emission: per chunk g the engine programs are
    attention(g) -> normalize(g) -> qkv(g+1) -> proj(g), so the PE FIFO
    never stalls at a chunk tail waiting for DVE (the proj matmuls sit
    behind qkv(g+1)), and DVE never stalls waiting for PE (normalize(g)
    sits before the g+1 bias-adds). All proj-psum evacuations are on DVE:
    putting any on ACT would block the strict-FIFO exp stream.
  - DMA issue cost (~0.6us per dma_start on the Sync sequencer) split
    across the Sync and GpSimd queues.
  - PE warm-up matmuls on the weight tile while x streams in, so the HAM
    clock gate reaches 8/8 before the first real matmul.
  - exp() is safe without max-subtraction: |scores| < 4 for this problem.

Layout notes (per core c, heads 2c and 2c+1):
  - xT   [1024, 2048]  x transposed (shared by all cores), bf16
  - wT   [1024, 384]   wqkv rows for (k,q,v) of this core's heads, transposed;
                       q-rows pre-scaled by 1/sqrt(64)=0.125 (exact)
  - qT/kT [128, 2048] in SBUF: rows = 2 heads x 64 dims, cols = tokens
  - v_sb [128, 16, 2, 65]: tokens on partitions, per token-tile / head the 64
    v dims plus a ones column (col 64) that makes row 64 of the AV psum
    accumulate the softmax denominator — no cross-partition reductions needed.
  - v bias is added after normalization (softmax rows sum to 1, so adding
    b_v to every v row adds exactly b_v to the attention output).
"""

from collections import deque

import numpy as np
import ml_dtypes

import concourse.bass as bass
import concourse.mybir as mybir
import concourse.tile as tile
from concourse import bacc
from concourse import bass_utils

T = 2048
C = 1024
H = 16
D = 64
N_CORES = 8
P = 128
NT = T // P          # 16 token tiles
NG = T // 512        # 4 column chunks of 512
NO = C // P          # 8 contraction subtiles

F32 = mybir.dt.float32
BF16 = mybir.dt.bfloat16
F16 = mybir.dt.float16
BF16_NP = ml_dtypes.bfloat16


def _build():
    nc = bacc.Bacc("TRN2", target_bir_lowering=False, debug=False,
                   num_devices=N_CORES)

    xT = nc.dram_tensor("xT", [C, T], BF16, kind="ExternalInput").ap()
    wT = nc.dram_tensor("wT", [C, 3 * P], BF16, kind="ExternalInput").ap()
    bqkv = nc.dram_tensor("bqkv", [P, 3], F32, kind="ExternalInput").ap()
    # pwl[i, o]: proj_w columns for this core's 128 attn rows, transposed
    pwl = nc.dram_tensor("pwl", [P, C], BF16, kind="ExternalInput").ap()
    mask01 = nc.dram_tensor("mask01", [P, 2, P], BF16, kind="ExternalInput").ap()
    # partial output: [1024 out-cols (as 8x128), 2048 tokens], fp16
    outP = nc.dram_tensor("outP", [C, T], F16, kind="ExternalOutput").ap()
    outP3 = outP.rearrange("(o p) t -> p o t", p=P)

    xT3 = xT.rearrange("(o p) t -> p o t", p=P)      # [128, 8, 2048]
    wT3 = wT.rearrange("(o p) j -> p o j", p=P)      # [128, 8, 384]

    # wqkv section order in wT columns: k, q, v
    JK, JQ, JV = 0, 1, 2

    with tile.TileContext(nc) as tc:
        with (
            tc.tile_pool(name="const", bufs=1) as constp,
            tc.tile_pool(name="big", bufs=1) as bigp,
            tc.tile_pool(name="work", bufs=4) as workp,
            tc.tile_pool(name="small", bufs=4) as smallp,
            tc.tile_pool(name="attn", bufs=2) as attnp,
            tc.tile_pool(name="outp", bufs=2) as outp,
            tc.tile_pool(name="ps_mm", bufs=2, space="PSUM") as ps_mm,
            tc.tile_pool(name="ps_sc", bufs=2, space="PSUM") as ps_sc,
            tc.tile_pool(name="ps_at", bufs=2, space="PSUM") as ps_at,
        ):
            # ---- PE warm-up on a memset tile: no DMA dependency, so the
            # HAM clock gate reaches 8/8 while the inputs are still in
            # flight and the first real matmul runs at 2.4 GHz ----
            warm_sb = constp.tile([P, 512], BF16, name="warm")
            nc.vector.memset(warm_sb[:], 1.0)
            wu = ps_sc.tile([P, 2, 512], F32, name="sc")
            for r in range(14):
                nc.tensor.matmul(wu[:, 0, :],
                                 lhsT=warm_sb[:, 0:P],
                                 rhs=warm_sb[:],
                                 start=True, stop=True)

            # ---- inputs; one batched dma_start per chunk (every dma_start
            # costs ~0.6us of sequencer time), spread over the Sync and
            # GpSimd queues; the first chunk of x is split across both
            # queues so qkv(0) can start earliest ----
            wT_sb = constp.tile([P, NO, 3 * P], BF16, name="wT")
            x_sb = bigp.tile([P, NO, T], BF16, name="x")
            bq_sb = constp.tile([P, 3], F32, name="bqkv")
            mask_sb = constp.tile([P, 2, P], BF16, name="mask")
            ones_sb = constp.tile([1, D], BF16, name="ones")
            nc.vector.memset(ones_sb[:], 1.0)
            pwl_sb = constp.tile([P, NO, P], BF16, name="pwl")
            nc.sync.dma_start(wT_sb[:], wT3[:])
            nc.gpsimd.dma_start(x_sb[:, 0:NO // 2, 0:512],
                                xT3[:, 0:NO // 2, 0:512])
            nc.sync.dma_start(x_sb[:, NO // 2:, 0:512],
                              xT3[:, NO // 2:, 0:512])
            nc.gpsimd.dma_start(bq_sb[:], bqkv)
            nc.gpsimd.dma_start(x_sb[:, :, 512:1024], xT3[:, :, 512:1024])
            nc.sync.dma_start(mask_sb[:], mask01)
            nc.sync.dma_start(x_sb[:, :, 1024:1536], xT3[:, :, 1024:1536])
            nc.gpsimd.dma_start(x_sb[:, :, 1536:2048], xT3[:, :, 1536:2048])
            nc.sync.dma_start(pwl_sb[:],
                              pwl.rearrange("p (o q) -> p o q", q=P))

            kT_sb = bigp.tile([P, T], BF16, name="kT")
            qT_sb = bigp.tile([P, T], BF16, name="qT")
            v_sb = bigp.tile([P, NT, 2, D + 1], BF16, name="v")
            nc.vector.memset(v_sb[:, :, :, D], 1.0)

            fillers = deque()

            def qkv_fillers(g):
                """Queue chunk g's qkv work as PE filler thunks (consumed
                inside the previous chunk's attention j-loop, where the PE
                otherwise idles waiting on ACT exp)."""
                cols = slice(g * 512, (g + 1) * 512)
                state = {}

                def kq_mm(j, dst, o):
                    def f():
                        if o == 0:
                            state[j] = ps_mm.tile([P, 512], F32, name="mm")
                        nc.tensor.matmul(
                            state[j][:],
                            lhsT=wT_sb[:, o, j * P:(j + 1) * P],
                            rhs=x_sb[:, o, cols],
                            start=(o == 0), stop=(o == NO - 1),
                        )
                        if o == NO - 1:
                            nc.vector.tensor_scalar_add(dst[:, cols],
                                                        state[j][:],
                                                        bq_sb[:, j:j + 1])
                    return f

                def v_mm(tt, o):
                    def f():
                        if tt == 0 and o == 0:
                            state[JV] = ps_mm.tile([P, 4, 2, D], F32,
                                                   name="mm")
                        t0 = g * 512 + tt * P
                        nc.tensor.matmul(
                            state[JV][:, tt, :, :],
                            lhsT=x_sb[:, o, t0:t0 + P],
                            rhs=wT_sb[:, o, JV * P:(JV + 1) * P],
                            start=(o == 0), stop=(o == NO - 1),
                        )
                        if tt == 3 and o == NO - 1:
                            nc.vector.tensor_copy(
                                out=v_sb[:, 4 * g:4 * g + 4, :, 0:D],
                                in_=state[JV][:])
                    return f

                for j, dst in ((JK, kT_sb), (JQ, qT_sb)):
                    for o in range(NO):
                        fillers.append(kq_mm(j, dst, o))
                for tt in range(4):
                    for o in range(NO):
                        fillers.append(v_mm(tt, o))

            def qkv(g):
                qkv_fillers(g)
                while fillers:
                    fillers.popleft()()

            def attention(g):
                # software-pipelined j-loop: AV(j) is emitted AFTER
                # scores(j+1), so the PE (strict FIFO) never stalls on the
                # ~1us exp(j) latency — by the time the PE reaches AV(j),
                # exp(j) ran on ACT during scores(j+1) + fillers.
                ats = [ps_at.tile([P, 512], F32, name="at") for _ in range(2)]
                nj = 4 * g + 4
                pend = None

                def av(j, t0, w_, e2):
                    for h in range(2):
                        nc.tensor.matmul(
                            ats[h][:D + 1, t0 - 512 * g:512],
                            lhsT=v_sb[:, j, h, :],
                            rhs=e2[:, h, 0:w_],
                            start=(j == 0), stop=(j == nj - 1),
                        )

                for j in range(nj):
                    t0 = 512 * g if j < 4 * g else P * j
                    w_ = 512 * (g + 1) - t0
                    sc2 = ps_sc.tile([P, 2, 512], F32, name="sc")
                    for h in range(2):
                        nc.tensor.matmul(
                            sc2[:, h, 0:w_],
                            lhsT=kT_sb[h * D:(h + 1) * D, j * P:(j + 1) * P],
                            rhs=qT_sb[h * D:(h + 1) * D, t0:t0 + w_],
                            start=True, stop=True,
                        )
                    if pend is not None:
                        av(*pend)
                    e2 = workp.tile([P, 2, 512], BF16, name="e2")
                    if w_ == 512:
                        nc.scalar.activation(e2[:], sc2[:],
                                             mybir.ActivationFunctionType.Exp)
                    else:
                        for h in range(2):
                            nc.scalar.activation(
                                e2[:, h, 0:w_], sc2[:, h, 0:w_],
                                mybir.ActivationFunctionType.Exp)
                    if j >= 4 * g:
                        # zero the strictly-upper (t<s) part of the diag block
                        nc.vector.tensor_mul(out=e2[:, :, 0:P],
                                             in0=e2[:, :, 0:P],
                                             in1=mask_sb[:])
                    pend = (j, t0, w_, e2)
                    for _ in range(3):
                        if fillers:
                            fillers.popleft()()
                av(*pend)
                return ats

            def normalize(g, ats):
                # rows 0..63 of ats[h] = unnormalized attnT, row 64 = denom
                attn_sb = attnp.tile([P, 512], BF16, name="attn")
                for h in range(2):
                    at = ats[h]
                    rs = smallp.tile([1, 512], BF16, name="rs")
                    nc.vector.tensor_copy(out=rs[:], in_=at[D:D + 1, :])
                    rb = ps_mm.tile([P, 512], F32, name="mm")
                    nc.tensor.matmul(rb[:D, :], lhsT=ones_sb[:], rhs=rs[:],
                                     start=True, stop=True)
                    rr = smallp.tile([D, 512], F32, name="rr")
                    nc.vector.reciprocal_approx_fast(out=rr[:], in_=rb[:D, :])
                    nc.vector.tensor_mul(out=attn_sb[h * D:(h + 1) * D, :],
                                         in0=at[0:D, :], in1=rr[:])
                # v bias (softmax rows sum to 1 -> plain add post-normalize)
                nc.vector.tensor_scalar_add(attn_sb[:], attn_sb[:],
                                            bq_sb[:, JV:JV + 1])
                return attn_sb

            def proj_fillers(g, attn_sb, last):
                cols = slice(g * 512, (g + 1) * 512)
                state = {}

                def pmm(ot):
                    def f():
                        if ot == 0:
                            state["ob"] = outp.tile([P, NO, 512], F16,
                                                    name="ob")
                        psp = ps_mm.tile([P, 512], F32, name="mm")
                        nc.tensor.matmul(psp[:], lhsT=pwl_sb[:, ot, :],
                                         rhs=attn_sb[:],
                                         start=True, stop=True)
                        ob = state["ob"]
                        # on the last chunk ACT has no more exps: split the
                        # evacuation between DVE and ACT to shorten the tail
                        if last and ot % 2 == 1:
                            nc.scalar.copy(ob[:, ot, :], psp[:])
                        else:
                            nc.vector.tensor_copy(out=ob[:, ot, :], in_=psp[:])
                        if ot == NO // 2 - 1:
                            nc.sync.dma_start(outP3[:, 0:NO // 2, cols],
                                              ob[:, 0:NO // 2, :])
                        elif ot == NO - 1:
                            nc.gpsimd.dma_start(outP3[:, NO // 2:, cols],
                                                ob[:, NO // 2:, :])
                    return f

                for ot in range(NO):
                    fillers.append(pmm(ot))

            # ---- software-pipelined chunk loop: chunk g's attention
            # j-loop absorbs proj(g-1) and qkv(g+1) as PE fillers ----
            qkv(0)
            for g in range(NG):
                if g + 1 < NG:
                    qkv_fillers(g + 1)
                ats = attention(g)
                while fillers:
                    fillers.popleft()()
                attn_sb = normalize(g, ats)
                proj_fillers(g, attn_sb, last=(g + 1 == NG))
            while fillers:
                fillers.popleft()()

    nc.compile()
    return nc


_NC = None
LAST_RESULT = None


def _get_nc():
    global _NC
    if _NC is None:
        _NC = _build()
    return _NC


def _prep_inputs(x, wqkv_w, wqkv_b, proj_w, proj_b):
    x = np.asarray(x, np.float32)
    wqkv_w = np.asarray(wqkv_w, np.float32)
    wqkv_b = np.asarray(wqkv_b, np.float32)
    proj_w = np.asarray(proj_w, np.float32)

    scale = np.float32(1.0 / np.sqrt(D))  # 0.125 exactly
    xT = np.ascontiguousarray(x.T).astype(BF16_NP)
    mask1 = np.triu(np.ones((P, P), np.float32))
    mask = np.ascontiguousarray(
        np.broadcast_to(mask1[:, None, :], (P, 2, P))).astype(BF16_NP)

    in_maps = []
    for c in range(N_CORES):
        qs = slice(P * c, P * (c + 1))
        ks = slice(C + P * c, C + P * (c + 1))
        vs = slice(2 * C + P * c, 2 * C + P * (c + 1))
        # column order in wT: k, q, v (q-rows pre-scaled)
        w_c = np.concatenate(
            [wqkv_w[ks], wqkv_w[qs] * scale, wqkv_w[vs]], axis=0)  # [384, 1024]
        b_c = np.concatenate(
            [wqkv_b[ks], wqkv_b[qs] * scale, wqkv_b[vs]])          # [384]
        in_maps.append({
            "xT": xT,
            "wT": np.ascontiguousarray(w_c.T).astype(BF16_NP),
            "bqkv": np.ascontiguousarray(b_c.reshape(3, P).T, dtype=np.float32),
            # proj_w columns for this core's attn rows, transposed -> [128, 1024]
            "pwl": np.ascontiguousarray(proj_w[:, qs].T).astype(BF16_NP),
            "mask01": mask,
        })
    return in_maps


def kernel(x, wqkv_w, wqkv_b, proj_w, proj_b):
    global LAST_RESULT
    nc = _get_nc()
    in_maps = _prep_inputs(x, wqkv_w, wqkv_b, proj_w, proj_b)
    res = bass_utils.run_bass_kernel_spmd(nc, in_maps,
                                          core_ids=list(range(N_CORES)))
    LAST_RESULT = res
    # unshard: the partials are sum-sharded over cores; reduce, transpose,
    # and apply the projection bias once.
    acc = res.results[0]["outP"].astype(np.float32)
    for c in range(1, N_CORES):
        acc = acc + res.results[c]["outP"].astype(np.float32)
    out = acc.T + np.asarray(proj_b, np.float32)[None, :]
    return np.ascontiguousarray(out).astype(np.float32)
